# revision 4
# baseline (speedup 1.0000x reference)
"""MoE top-2 routed linear (nn_MoELinear) on 8 Trainium2 NeuronCores.

Strategy (expert parallelism, per the sharding hint):
  - Gating (tiny: [N,1024]x[1024,8] matmul + top-2 + softmax) is computed on
    host with jax-CPU, replicating the reference op-for-op so the top-2
    decisions match the reference bitwise.
  - Tokens are dispatched (gathered) per expert on host; core e receives the
    tokens routed to expert e (padded to a common capacity C), expert e's
    weights pre-transposed to [d_in, d_out], and the per-token gate weight.
  - Each core computes Y_e = (X_e @ We[e].T) * w_e[:, None]  -- a dense
    [C,1024]x[1024,4096] matmul with the gate scale applied on-chip during
    PSUM eviction.  Expert weights are cached entirely in SBUF.
  - Host combines: out[token] = sum of its (two) expert contributions.
"""

import os

import numpy as np

NUM_CORES = 8
TOP_K = 2
P = 128  # partitions
N_TILE = 512  # psum free-dim tile (one bank of fp32)

# matmul dtype knob: "float32" (exact, 4 cyc/row), "float32r" (full rate),
# "bfloat16" (full rate, halves input DMA)
MM_DTYPE = os.environ.get("MOE_MM_DTYPE", "float32")
# enable NTFF tracing (sets LAST_RUN_INFO["exec_time_ns"])
TRACE = os.environ.get("MOE_TRACE", "0") == "1"

LAST_RUN_INFO = {}


def _routing(x_flat, Wg, bg):
    """Replicate the reference gating bitwise on jax-CPU; numpy fallback."""
    try:
        import jax
        import jax.numpy as jnp

        with jax.default_device(jax.devices("cpu")[0]):
            xf = jnp.asarray(x_flat)
            gate_logits = xf @ jnp.asarray(Wg).T + jnp.asarray(bg)
            top_w, top_idx = jax.lax.top_k(gate_logits, TOP_K)
            top_w = jax.nn.softmax(top_w, axis=-1)
            return np.asarray(top_idx), np.asarray(top_w)
    except Exception:
        logits = x_flat @ Wg.T + bg
        top_idx = np.argsort(-logits, axis=1, kind="stable")[:, :TOP_K]
        top_v = np.take_along_axis(logits, top_idx, axis=1)
        e = np.exp(top_v - top_v.max(axis=1, keepdims=True))
        top_w = e / e.sum(axis=1, keepdims=True)
        return top_idx, top_w.astype(np.float32)


def _build_program(C, CIN, DOUT, mm_dtype):
    """One-expert program: y[C,DOUT] = (xt[CIN,C].T @ wt[CIN,DOUT]) * sc."""
    import concourse.mybir as mybir
    import concourse.tile as tile
    from concourse import bacc

    f32 = mybir.dt.float32
    if mm_dtype == "bfloat16":
        io_dt = mybir.dt.bfloat16
        mm_dt = mybir.dt.bfloat16
    elif mm_dtype == "float32r":
        io_dt = f32
        mm_dt = mybir.dt.float32r
    else:
        io_dt = f32
        mm_dt = f32

    KT = CIN // P
    MT = C // P
    NT = DOUT // N_TILE

    nc = bacc.Bacc()
    xt = nc.declare_dram_parameter("xt", [CIN, C], io_dt, isOutput=False)
    wt = nc.declare_dram_parameter("wt", [CIN, DOUT], io_dt, isOutput=False)
    sc = nc.declare_dram_parameter("sc", [C, 1], f32, isOutput=False)
    y = nc.declare_dram_parameter("y", [C, DOUT], f32, isOutput=True)

    with tile.TileContext(nc) as tc:
        with (
            tc.tile_pool(name="wpool", bufs=1) as wpool,
            tc.tile_pool(name="xpool", bufs=3) as xpool,
            tc.tile_pool(name="spool", bufs=3) as spool,
            tc.tile_pool(name="opool", bufs=4) as opool,
            tc.tile_pool(name="pspool", bufs=4, space="PSUM") as pspool,
        ):
            # cache all expert weights in SBUF, one [P, DOUT] tile per k-chunk
            wtiles = []
            for k in range(KT):
                wtile = wpool.tile([P, DOUT], io_dt, tag=f"w{k}")
                nc.sync.dma_start(out=wtile[:], in_=wt[k * P : (k + 1) * P, :])
                wtiles.append(wtile)

            for m in range(MT):
                # lhsT tiles for this token block: [P(cin chunk), P(tokens)] x KT
                xtile = xpool.tile([P, KT * P], io_dt)
                for k in range(KT):
                    nc.sync.dma_start(
                        out=xtile[:, k * P : (k + 1) * P],
                        in_=xt[k * P : (k + 1) * P, m * P : (m + 1) * P],
                    )
                stile = spool.tile([P, 1], f32)
                nc.sync.dma_start(out=stile[:], in_=sc[m * P : (m + 1) * P, :])

                for n in range(NT):
                    psum = pspool.tile([P, N_TILE], f32)
                    for k in range(KT):
                        nc.tensor.matmul(
                            psum[:],
                            lhsT=xtile[:, k * P : (k + 1) * P].bitcast(mm_dt),
                            rhs=wtiles[k][:, n * N_TILE : (n + 1) * N_TILE].bitcast(
                                mm_dt
                            ),
                            start=(k == 0),
                            stop=(k == KT - 1),
                        )
                    otile = opool.tile([P, N_TILE], f32)
                    nc.scalar.activation(
                        otile[:],
                        psum[:],
                        mybir.ActivationFunctionType.Copy,
                        scale=stile[:],
                    )
                    nc.sync.dma_start(
                        out=y[m * P : (m + 1) * P, n * N_TILE : (n + 1) * N_TILE],
                        in_=otile[:],
                    )
    nc.finalize()
    return nc


def kernel(x, We, Wg, bg):
    from concourse.bass_utils import run_bass_kernel_spmd

    B, T, CIN = x.shape
    E, DOUT, _ = We.shape
    N = B * T
    x_flat = np.ascontiguousarray(x.reshape(N, CIN), dtype=np.float32)

    top_idx, top_w = _routing(x_flat, Wg, bg)

    # dispatch: token lists per expert
    idx_e = []
    w_e = []
    for e in range(E):
        sel0 = top_idx[:, 0] == e
        sel1 = top_idx[:, 1] == e
        rows = np.nonzero(sel0 | sel1)[0]
        w = np.where(sel0[rows], top_w[rows, 0], top_w[rows, 1]).astype(np.float32)
        idx_e.append(rows)
        w_e.append(w)

    cmax = max(len(r) for r in idx_e)
    C = max(P, ((cmax + P - 1) // P) * P)

    io_np = np.float32
    if MM_DTYPE == "bfloat16":
        import ml_dtypes

        io_np = ml_dtypes.bfloat16

    in_maps = []
    for e in range(E):
        ce = len(idx_e[e])
        xg = np.zeros((C, CIN), np.float32)
        xg[:ce] = x_flat[idx_e[e]]
        xt = np.ascontiguousarray(xg.T).astype(io_np)
        wt = np.ascontiguousarray(We[e].T).astype(io_np)
        sc = np.zeros((C, 1), np.float32)
        sc[:ce, 0] = w_e[e]
        in_maps.append({"xt": xt, "wt": wt, "sc": sc})

    nc = _build_program(C, CIN, DOUT, MM_DTYPE)
    res = run_bass_kernel_spmd(nc, in_maps, list(range(NUM_CORES)), trace=TRACE)

    LAST_RUN_INFO.clear()
    LAST_RUN_INFO.update(
        exec_time_ns=res.exec_time_ns,
        mean_exec_time_ns=res.mean_exec_time_ns,
        max_exec_time_core_id=res.max_exec_time_core_id,
        profile_json=res.profile_json,
    )

    out = np.zeros((N, DOUT), np.float32)
    for e in range(E):
        ye = res.results[e]["y"]
        out[idx_e[e]] += ye[: len(idx_e[e])]
    return out.reshape(B, T, DOUT)


# revision 5
# speedup vs baseline: 3.2205x; 3.2205x over previous
"""MoE top-2 routed linear (nn_MoELinear) on 8 Trainium2 NeuronCores.

Strategy (expert parallelism, per the sharding hint):
  - Gating (tiny: [N,1024]x[1024,8] matmul + top-2 + softmax) is computed on
    host with jax-CPU, replicating the reference op-for-op so the top-2
    decisions match the reference bitwise.
  - Tokens are dispatched (gathered) per expert on host; core e receives the
    tokens routed to expert e (padded to a common capacity C), expert e's
    weights pre-transposed to [d_in, d_out], and the per-token gate weight.
  - Each core computes Y_e = (X_e @ We[e].T) * w_e[:, None]  -- a dense
    [C,1024]x[1024,4096] matmul with the gate scale applied on-chip during
    PSUM eviction.  Expert weights are cached entirely in SBUF.
  - Host combines: out[token] = sum of its (two) expert contributions.
"""

import os

import numpy as np

NUM_CORES = 8
TOP_K = 2
P = 128  # partitions
N_TILE = 512  # psum free-dim tile (one bank of fp32)

# matmul dtype knob: "float32" (exact, 4 cyc/row), "float32r" (full rate),
# "bfloat16" (full rate, halves input DMA)
MM_DTYPE = os.environ.get("MOE_MM_DTYPE", "float32")
# enable NTFF tracing (sets LAST_RUN_INFO["exec_time_ns"])
TRACE = os.environ.get("MOE_TRACE", "0") == "1"

LAST_RUN_INFO = {}


def _routing(x_flat, Wg, bg):
    """Replicate the reference gating bitwise on jax-CPU; numpy fallback."""
    try:
        import jax
        import jax.numpy as jnp

        with jax.default_device(jax.devices("cpu")[0]):
            xf = jnp.asarray(x_flat)
            gate_logits = xf @ jnp.asarray(Wg).T + jnp.asarray(bg)
            top_w, top_idx = jax.lax.top_k(gate_logits, TOP_K)
            top_w = jax.nn.softmax(top_w, axis=-1)
            return np.asarray(top_idx), np.asarray(top_w)
    except Exception:
        logits = x_flat @ Wg.T + bg
        top_idx = np.argsort(-logits, axis=1, kind="stable")[:, :TOP_K]
        top_v = np.take_along_axis(logits, top_idx, axis=1)
        e = np.exp(top_v - top_v.max(axis=1, keepdims=True))
        top_w = e / e.sum(axis=1, keepdims=True)
        return top_idx, top_w.astype(np.float32)


def _build_program(C, CIN, DOUT, mm_dtype):
    """One-expert program: y[C,DOUT] = (xt[CIN,C].T @ wt[CIN,DOUT]) * sc."""
    import concourse.mybir as mybir
    import concourse.tile as tile
    from concourse import bacc

    f32 = mybir.dt.float32
    if mm_dtype == "bfloat16":
        io_dt = mybir.dt.bfloat16
        mm_dt = mybir.dt.bfloat16
    elif mm_dtype == "float32r":
        io_dt = mybir.dt.float32r
        mm_dt = mybir.dt.float32r
    else:
        io_dt = f32
        mm_dt = f32

    KT = CIN // P
    MT = C // P
    NT = DOUT // N_TILE

    nc = bacc.Bacc()
    xt = nc.declare_dram_parameter("xt", [CIN, C], io_dt, isOutput=False)
    wt = nc.declare_dram_parameter("wt", [CIN, DOUT], io_dt, isOutput=False)
    sc = nc.declare_dram_parameter("sc", [C, 1], f32, isOutput=False)
    y = nc.declare_dram_parameter("y", [C, DOUT], f32, isOutput=True)

    with tile.TileContext(nc) as tc:
        with (
            tc.tile_pool(name="wpool", bufs=1) as wpool,
            tc.tile_pool(name="xpool", bufs=3) as xpool,
            tc.tile_pool(name="spool", bufs=3) as spool,
            tc.tile_pool(name="opool", bufs=4) as opool,
            tc.tile_pool(name="pspool", bufs=4, space="PSUM") as pspool,
        ):
            # cache all expert weights in SBUF, one [P, DOUT] tile per k-chunk
            wtiles = []
            for k in range(KT):
                wtile = wpool.tile([P, DOUT], io_dt, tag=f"w{k}")
                nc.sync.dma_start(out=wtile[:], in_=wt[k * P : (k + 1) * P, :])
                wtiles.append(wtile)

            for m in range(MT):
                # lhsT tiles for this token block: [P(cin chunk), P(tokens)] x KT
                xtile = xpool.tile([P, KT * P], io_dt)
                for k in range(KT):
                    nc.sync.dma_start(
                        out=xtile[:, k * P : (k + 1) * P],
                        in_=xt[k * P : (k + 1) * P, m * P : (m + 1) * P],
                    )
                stile = spool.tile([P, 1], f32)
                nc.sync.dma_start(out=stile[:], in_=sc[m * P : (m + 1) * P, :])

                for n in range(NT):
                    psum = pspool.tile([P, N_TILE], f32)
                    for k in range(KT):
                        nc.tensor.matmul(
                            psum[:],
                            lhsT=xtile[:, k * P : (k + 1) * P].bitcast(mm_dt),
                            rhs=wtiles[k][:, n * N_TILE : (n + 1) * N_TILE].bitcast(
                                mm_dt
                            ),
                            start=(k == 0),
                            stop=(k == KT - 1),
                        )
                    otile = opool.tile([P, N_TILE], f32)
                    nc.scalar.activation(
                        otile[:],
                        psum[:],
                        mybir.ActivationFunctionType.Copy,
                        scale=stile[:],
                    )
                    nc.sync.dma_start(
                        out=y[m * P : (m + 1) * P, n * N_TILE : (n + 1) * N_TILE],
                        in_=otile[:],
                    )
    nc.finalize()
    return nc


def kernel(x, We, Wg, bg):
    from concourse.bass_utils import run_bass_kernel_spmd

    B, T, CIN = x.shape
    E, DOUT, _ = We.shape
    N = B * T
    x_flat = np.ascontiguousarray(x.reshape(N, CIN), dtype=np.float32)

    top_idx, top_w = _routing(x_flat, Wg, bg)

    # dispatch: token lists per expert
    idx_e = []
    w_e = []
    for e in range(E):
        sel0 = top_idx[:, 0] == e
        sel1 = top_idx[:, 1] == e
        rows = np.nonzero(sel0 | sel1)[0]
        w = np.where(sel0[rows], top_w[rows, 0], top_w[rows, 1]).astype(np.float32)
        idx_e.append(rows)
        w_e.append(w)

    cmax = max(len(r) for r in idx_e)
    C = max(P, ((cmax + P - 1) // P) * P)

    io_np = np.float32
    if MM_DTYPE == "bfloat16":
        import ml_dtypes

        io_np = ml_dtypes.bfloat16

    in_maps = []
    for e in range(E):
        ce = len(idx_e[e])
        xg = np.zeros((C, CIN), np.float32)
        xg[:ce] = x_flat[idx_e[e]]
        xt = np.ascontiguousarray(xg.T).astype(io_np)
        wt = np.ascontiguousarray(We[e].T).astype(io_np)
        sc = np.zeros((C, 1), np.float32)
        sc[:ce, 0] = w_e[e]
        in_maps.append({"xt": xt, "wt": wt, "sc": sc})

    nc = _build_program(C, CIN, DOUT, MM_DTYPE)
    res = run_bass_kernel_spmd(nc, in_maps, list(range(NUM_CORES)), trace=TRACE)

    LAST_RUN_INFO.clear()
    LAST_RUN_INFO.update(
        exec_time_ns=res.exec_time_ns,
        mean_exec_time_ns=res.mean_exec_time_ns,
        max_exec_time_core_id=res.max_exec_time_core_id,
        profile_json=res.profile_json,
    )

    out = np.zeros((N, DOUT), np.float32)
    for e in range(E):
        ye = res.results[e]["y"]
        out[idx_e[e]] += ye[: len(idx_e[e])]
    return out.reshape(B, T, DOUT)
